# revision 9
# baseline (speedup 1.0000x reference)
"""Trainium2 Bass kernel for an 8-layer decoder transformer w/ LM head + CE loss.

Sharding (8 NeuronCores, Megatron TP):
  - attention heads:   2 heads / core  (H=16)
  - MLP intermediate:  512 rows / core (DF=4096)
  - lm head vocab:     4000 cols / core (V=32000)
  - residual stream h: replicated, fp32, feature-major [D, B*S] in SBUF
  - 2 AllReduces / layer (Wo out, W2 out), fp32, split in 2 half-batch chunks

Activations feed matmuls in bf16; PSUM accumulation fp32. LayerNorm mean is
tracked as a running column-sum (rank-1 correction folded into the next
matmul); variance via bf16 squares + ones-matmul column sums.
"""
import sys, os
sys.path.insert(0, "/opt/trn_rl_repo")

import numpy as np
import ml_dtypes

_BF16 = ml_dtypes.bfloat16

# ---- model dims (hardcoded per problem spec) ----
V, D, H, BS, L = 32000, 1024, 16, 2048, 8
DH = D // H          # 64
DF = 4 * D           # 4096
B, T, M = 2, 1024, 64
S = M + T            # 1088
NTOK = B * S         # 2176
NC = 8               # cores
HPC = H // NC        # 2 heads/core
DHC = HPC * DH       # 128
DFC = DF // NC       # 512
VC = V // NC         # 4000
TT = B * T           # 2048 lm tokens
EPS = 1e-5
DCH = D // 128       # 8 d-chunks
QT_PER_B = 9         # q tiles per batch (8x128 + 64)

def _half_chunks(b):
    base = b * S
    return [(base, 512), (base + 512, 512), (base + 1024, 64)]

# final-LN chunks: (orig_start, width, compact_start)
FLN_CHUNKS = [(64, 512, 0), (576, 512, 512), (1152, 512, 1024), (1664, 512, 1536)]

DEBUG = bool(int(os.environ.get("KERNEL_DEBUG", "0")))

_cache = {}


def _build():
    import concourse.bass as bass
    import concourse.tile as tile
    import contextlib
    from concourse import bacc, mybir
    from concourse.masks import make_identity, make_causal_mask

    f32, bf16 = mybir.dt.float32, mybir.dt.bfloat16
    A = mybir.AluOpType
    AF = mybir.ActivationFunctionType

    nc = bacc.Bacc("TRN2", target_bir_lowering=False, debug=False,
                   enable_asserts=False, num_devices=NC)

    def din(name, shape, dt=bf16):
        return nc.dram_tensor(name, shape, dt, kind="ExternalInput").ap()

    h0_d = din("h0", [128, DCH, NTOK], f32)
    h0cs_d = din("h0cs", [1, NTOK], f32)
    wq_d = din("wq", [L, 128, DCH, DHC])
    wk_d = din("wk", [L, 128, DCH, DHC])
    wv_d = din("wv", [L, 128, DCH, DHC])
    wo_d = din("wo", [L, 128, D])
    w1_d = din("w1", [L, 128, DCH, DFC])
    w2_d = din("w2", [L, 128, 4, D])
    worow_d = din("worow", [L, 128, 1])
    w2row_d = din("w2row", [L, 128, 4])
    sb2_d = din("sb2", [L, 1, 1])
    cwqkv_d = din("cwqkv", [L, 1, 3 * DHC])
    cbqkv_d = din("cbqkv", [L, 1, 3 * DHC])
    cw1_d = din("cw1", [L, 1, DFC])
    cb1_d = din("cb1", [L, 1, DFC])
    cb2_d = din("cb2", [L, 1, D])
    wlm_d = din("wlm", [128, DCH, VC])
    cwlm_d = din("cwlm", [1, VC])
    cblm_d = din("cblm", [1, VC])

    logits_d = nc.dram_tensor("logits", [TT, VC], f32, kind="ExternalOutput").ap()
    sumexp_d = nc.dram_tensor("sumexp", [TT // 128, 128], f32, kind="ExternalOutput").ap()
    if DEBUG:
        dbg_h1_d = nc.dram_tensor("dbg_h1", [128, DCH, NTOK], f32, kind="ExternalOutput").ap()
        dbg_h2_d = nc.dram_tensor("dbg_h2", [128, DCH, NTOK], f32, kind="ExternalOutput").ap()
        dbg_q_d = nc.dram_tensor("dbg_q", [128, NTOK], bf16, kind="ExternalOutput").ap()
        dbg_o_d = nc.dram_tensor("dbg_o", [128, NTOK], bf16, kind="ExternalOutput").ap()

    RG = [list(range(NC))]

    tc_ref = {}
    with tile.TileContext(nc) as tc:
        tc_ref["tc"] = tc
        with contextlib.ExitStack() as ctx:
            sbp = ctx.enter_context(tc.tile_pool(name="persist", bufs=1))
            rowp = ctx.enter_context(tc.tile_pool(name="rows", bufs=2))
            ps512 = ctx.enter_context(tc.tile_pool(name="ps512", bufs=2, space="PSUM"))
            psbig = ctx.enter_context(tc.tile_pool(name="psbig", bufs=2, space="PSUM"))
            dramp = ctx.enter_context(tc.tile_pool(name="drp", bufs=2, space="DRAM"))

            # ---- persistent state ----
            h = sbp.tile([128, DCH, NTOK], f32, name="h")
            for dc in range(DCH):
                nc.sync.dma_start(h[:, dc, :], h0_d[:, dc, :])
            colsum = sbp.tile([1, NTOK], f32, name="colsum")
            nc.sync.dma_start(colsum[:], h0cs_d[:])

            epsb = sbp.tile([1, 1], f32, name="epsb")
            nc.vector.memset(epsb[:], float(EPS * 1024.0 * 1024.0))
            ones_bf = sbp.tile([128, 1], bf16, name="ones_bf")
            nc.vector.memset(ones_bf[:], 1.0)
            ones1024 = sbp.tile([1, 128], bf16, name="ones1024")
            nc.vector.memset(ones1024[:], 1024.0)
            onesrow = sbp.tile([1, 512], bf16, name="onesrow")
            nc.vector.memset(onesrow[:], 1.0)
            ident = sbp.tile([128, 128], bf16, name="ident")
            make_identity(nc, ident[:])
            causU = sbp.tile([128, 128], bf16, name="causU")
            make_causal_mask(nc, causU[:], mask_val=-1e30)

            qT = sbp.tile([128, NTOK], bf16, name="qT")
            kT = sbp.tile([128, NTOK], bf16, name="kT")
            vTok = sbp.tile([128, 2 * QT_PER_B, DHC], bf16, name="vTok")
            oT = sbp.tile([128, NTOK], bf16, name="oT")

            def layer_norm_chunk(o0, w, sqpool, actpool, act_tag="act",
                                 act_bufs=None, negmr_dst=None):
                """LN stats+apply for tokens [o0, o0+w). Returns (act, negmr)
                with act [128, DCH, w] bf16, negmr [1, w] bf16 (local idx)."""
                ssq = ps512.tile([128, 512], f32, tag="ps512", name="ssq")
                for q4 in range(2):
                    sq = sqpool.tile([128, 4, 512], bf16, tag="sq", bufs=2, name="sq")
                    nc.vector.tensor_tensor(
                        sq[:, :, :w], h[:, 4 * q4:4 * q4 + 4, o0:o0 + w],
                        h[:, 4 * q4:4 * q4 + 4, o0:o0 + w], A.mult)
                    for i in range(4):
                        dc = 4 * q4 + i
                        nc.tensor.matmul(ssq[0:1, :w], ones_bf[:], sq[:, i, :w],
                                         start=(dc == 0), stop=(dc == DCH - 1))
                m2 = rowp.tile([1, 512], f32, tag="m2", name="m2")
                nc.vector.tensor_tensor(m2[:, :w], colsum[0:1, o0:o0 + w],
                                        colsum[0:1, o0:o0 + w], A.mult)
                tv = rowp.tile([1, 512], f32, tag="tv", name="tv")
                nc.vector.scalar_tensor_tensor(tv[:, :w], ssq[0:1, :w], 1024.0,
                                               m2[:, :w], A.mult, A.subtract)
                sd = rowp.tile([1, 512], f32, tag="sd", name="sd")
                nc.scalar.activation(sd[:, :w], tv[:, :w], AF.Sqrt, bias=epsb[:])
                r0 = rowp.tile([1, 512], bf16, tag="r0", name="r0")
                if negmr_dst is None:
                    negmr_t = rowp.tile([1, 512], bf16, tag="negmr", name="negmr")
                    negmr = negmr_t[0:1, :w]
                else:
                    negmr = negmr_dst
                with nc.allow_low_precision(reason="bf16 rstd for bf16 matmul input"):
                    nc.vector.reciprocal(r0[:, :w], sd[:, :w])
                    nc.vector.scalar_tensor_tensor(negmr,
                                                   colsum[0:1, o0:o0 + w], -1.0,
                                                   r0[:, :w], A.mult, A.mult)
                rb = ps512.tile([128, 512], f32, tag="ps512", name="rb")
                nc.tensor.matmul(rb[:, :w], ones1024[:], r0[0:1, :w],
                                 start=True, stop=True)
                act = actpool.tile([128, DCH, 512], bf16, tag=act_tag,
                                   bufs=act_bufs, name=act_tag)
                for q4 in range(2):
                    nc.vector.tensor_tensor(
                        act[:, 4 * q4:4 * q4 + 4, :w],
                        h[:, 4 * q4:4 * q4 + 4, o0:o0 + w],
                        rb[:, None, :w].to_broadcast((128, 4, w)), A.mult)
                return act, negmr

            with tc.tile_pool(name="wpool", bufs=1) as wpool, \
                 tc.tile_pool(name="acts", bufs=3) as actp, \
                 tc.tile_pool(name="scratch", bufs=3) as scrp, \
                 tc.tile_pool(name="attn", bufs=2) as attp:
                def dma_w(shape, src, tag, bufs=1):
                    t = wpool.tile(shape, bf16, tag=tag, bufs=bufs, name=tag)
                    nc.sync.dma_start(t[:], src)
                    return t

                for l in range(L):
                    wq = dma_w([128, DCH, DHC], wq_d[l], "wq", bufs=2)
                    wk = dma_w([128, DCH, DHC], wk_d[l], "wk", bufs=2)
                    wv = dma_w([128, DCH, DHC], wv_d[l], "wv", bufs=2)
                    wo = dma_w([128, D], wo_d[l], "wo")
                    w1 = dma_w([128, DCH, DFC], w1_d[l], "w1")
                    w2 = dma_w([128, 4, D], w2_d[l], "w2")
                    worow = dma_w([128, 1], worow_d[l], "worow")
                    w2row = dma_w([128, 4], w2row_d[l], "w2row")
                    sb2 = dma_w([1, 1], sb2_d[l], "sb2")
                    cwqkv = dma_w([1, 3 * DHC], cwqkv_d[l], "cwqkv")
                    cbqkv = dma_w([1, 3 * DHC], cbqkv_d[l], "cbqkv")
                    cw1 = dma_w([1, DFC], cw1_d[l], "cw1")
                    cb1 = dma_w([1, DFC], cb1_d[l], "cb1")
                    cb2 = dma_w([1, D], cb2_d[l], "cb2")

                    # ===== LN1 + QKV =====
                    for b in range(B):
                        for (o0, w) in _half_chunks(b):
                            act, negmr = layer_norm_chunk(o0, w, scrp, actp)
                            # q, k (feature-major out)
                            for ti, (wt, dst) in enumerate(((wq, qT), (wk, kT))):
                                ps = ps512.tile([128, 512], f32, tag="ps512", name="qk")
                                for dc in range(DCH):
                                    nc.tensor.matmul(ps[:, :w], wt[:, dc, :],
                                                     act[:, dc, :w],
                                                     start=(dc == 0), stop=False)
                                cs = slice(ti * DHC, (ti + 1) * DHC)
                                nc.tensor.matmul(ps[:, :w], cwqkv[0:1, cs],
                                                 negmr,
                                                 start=False, stop=False)
                                nc.tensor.matmul(ps[:, :w], cbqkv[0:1, cs],
                                                 onesrow[0:1, :w],
                                                 start=False, stop=True)
                                nc.scalar.copy(dst[:, o0:o0 + w], ps[:, :w])
                            # v (token-major out)
                            loc0 = o0 - b * S
                            for j in range(max(1, w // 128)):
                                tw = min(128, w - j * 128)
                                jt = b * QT_PER_B + loc0 // 128 + j
                                ps = ps512.tile([128, 512], f32, tag="ps512", name="vps")
                                for dc in range(DCH):
                                    nc.tensor.matmul(
                                        ps[:tw, :DHC],
                                        act[:, dc, j * 128:j * 128 + tw],
                                        wv[:, dc, :], start=(dc == 0), stop=False)
                                nc.tensor.matmul(ps[:tw, :DHC],
                                                 negmr[0:1, j * 128:j * 128 + tw],
                                                 cwqkv[0:1, 2 * DHC:3 * DHC],
                                                 start=False, stop=False)
                                nc.tensor.matmul(ps[:tw, :DHC],
                                                 onesrow[0:1, :tw],
                                                 cbqkv[0:1, 2 * DHC:3 * DHC],
                                                 start=False, stop=True)
                                nc.scalar.copy(vTok[:tw, jt, :], ps[:tw, :DHC])

                    # ===== attention =====
                    for b in range(B):
                        base = b * S
                        for qi in range(QT_PER_B):
                            qrows = 128 if qi < 8 else 64
                            q0 = base + 128 * qi
                            klen = min(S, 128 * (qi + 1))
                            o_ps = ps512.tile([128, 512], f32, tag="ps512", name="ops")
                            for hh in range(HPC):
                                hp = slice(64 * hh, 64 * hh + 64)
                                sc = psbig.tile([128, S], f32, tag="scps", name="scps")
                                nk = (klen + 511) // 512
                                for c in range(nk):
                                    k0, kw = 512 * c, min(512, klen - 512 * c)
                                    # each 512-col chunk is its own psum bank:
                                    # needs its own start=True to clear
                                    # stale has_written state
                                    nc.tensor.matmul(sc[:qrows, k0:k0 + kw],
                                                     qT[hp, q0:q0 + qrows],
                                                     kT[hp, base + k0:base + k0 + kw],
                                                     start=True,
                                                     stop=(c != nk - 1))
                                nc.tensor.matmul(sc[:qrows, 128 * qi:128 * qi + qrows],
                                                 ident[:, :qrows], causU[:, :qrows],
                                                 start=False, stop=True)
                                at = attp.tile([128, 9 * 128], bf16, tag="attn", name="at")
                                sume = scrp.tile([128, 1], f32, tag="sume", name="sume")
                                nc.scalar.activation(at[:qrows, :klen], sc[:qrows, :klen],
                                                     AF.Exp, accum_out=sume[:qrows, :])
                                rec = scrp.tile([128, 1], f32, tag="rec", name="rec")
                                nc.vector.reciprocal(rec[:qrows, :], sume[:qrows, :])
                                nc.vector.tensor_scalar_mul(at[:qrows, :klen],
                                                            at[:qrows, :klen],
                                                            rec[:qrows, :])
                                for kt in range(qi + 1):
                                    kw = min(128, S - 128 * kt)
                                    atT = scrp.tile([128, 128], bf16, tag="atT", name="atT")
                                    nc.sync.dma_start(atT[:], at[:, 128 * kt:128 * (kt + 1)],
                                                      transpose=True)
                                    nc.tensor.matmul(
                                        o_ps[64 * hh:64 * hh + 64, :qrows],
                                        vTok[:kw, b * QT_PER_B + kt, hp],
                                        atT[:kw, :qrows],
                                        start=(kt == 0), stop=(kt == qi),
                                        tile_position=(0, 64 * hh))
                            nc.scalar.copy(oT[:, q0:q0 + qrows], o_ps[:, :qrows])

                    # ===== Wo partial + AR + residual, per half =====
                    for b in range(B):
                        arin = dramp.tile([1025, S], f32, tag="arin", name="arin")
                        arout = dramp.tile([1025, S], f32, tag="arout", name="arout",
                                           addr_space="Shared")
                        for (o0, w) in _half_chunks(b):
                            loc0 = o0 - b * S
                            for dc in range(DCH):
                                ps = ps512.tile([128, 512], f32, tag="ps512", name="wops")
                                nc.tensor.matmul(ps[:, :w], wo[:, 128 * dc:128 * (dc + 1)],
                                                 oT[:, o0:o0 + w], start=True, stop=True)
                                ds = scrp.tile([128, 512], f32, tag="ds", name="ds")
                                nc.scalar.copy(ds[:, :w], ps[:, :w])
                                nc.sync.dma_start(arin[128 * dc:128 * (dc + 1), loc0:loc0 + w],
                                                  ds[:, :w])
                            ps = ps512.tile([128, 512], f32, tag="ps512", name="worow_ps")
                            nc.tensor.matmul(ps[0:1, :w], worow[:], oT[:, o0:o0 + w],
                                             start=True, stop=True)
                            rs = rowp.tile([1, 512], f32, tag="rs", name="rs")
                            nc.scalar.copy(rs[:, :w], ps[0:1, :w])
                            nc.sync.dma_start(arin[1024:1025, loc0:loc0 + w], rs[:, :w])
                        nc.gpsimd.collective_compute(
                            "AllReduce", A.add, replica_groups=RG,
                            ins=[arin[:].opt()], outs=[arout[:].opt()])
                        for (o0, w) in _half_chunks(b):
                            loc0 = o0 - b * S
                            for dc in range(DCH):
                                ds = scrp.tile([128, 512], f32, tag="ds", name="dsr")
                                nc.sync.dma_start(ds[:, :w],
                                                  arout[128 * dc:128 * (dc + 1), loc0:loc0 + w])
                                nc.vector.tensor_tensor(h[:, dc, o0:o0 + w],
                                                        h[:, dc, o0:o0 + w],
                                                        ds[:, :w], A.add)
                            rs = rowp.tile([1, 512], f32, tag="rs", name="rsr")
                            nc.sync.dma_start(rs[:, :w], arout[1024:1025, loc0:loc0 + w])
                            nc.vector.tensor_tensor(colsum[0:1, o0:o0 + w],
                                                    colsum[0:1, o0:o0 + w],
                                                    rs[:, :w], A.add)

                    if DEBUG and l == 0:
                        for dc in range(DCH):
                            nc.sync.dma_start(dbg_h1_d[:, dc, :], h[:, dc, :])
                        nc.sync.dma_start(dbg_q_d[:], qT[:])
                        nc.sync.dma_start(dbg_o_d[:], oT[:])

                    # ===== LN2 + MLP + AR + residual, per half =====
                    for b in range(B):
                        arin = dramp.tile([1025, S], f32, tag="arin", name="arin2")
                        arout = dramp.tile([1025, S], f32, tag="arout", name="arout2",
                                           addr_space="Shared")
                        for (o0, w) in _half_chunks(b):
                            loc0 = o0 - b * S
                            act, negmr = layer_norm_chunk(o0, w, scrp, actp)
                            g = scrp.tile([128, 4, 512], bf16, tag="g", bufs=2, name="g")
                            for fc in range(4):
                                ps = ps512.tile([128, 512], f32, tag="ps512", name="w1ps")
                                for dc in range(DCH):
                                    nc.tensor.matmul(ps[:, :w], w1[:, dc, 128 * fc:128 * (fc + 1)],
                                                     act[:, dc, :w],
                                                     start=(dc == 0), stop=False)
                                fs = slice(128 * fc, 128 * (fc + 1))
                                nc.tensor.matmul(ps[:, :w], cw1[0:1, fs],
                                                 negmr, start=False, stop=False)
                                nc.tensor.matmul(ps[:, :w], cb1[0:1, fs],
                                                 onesrow[0:1, :w], start=False, stop=True)
                                nc.scalar.activation(g[:, fc, :w], ps[:, :w],
                                                     AF.Gelu_apprx_tanh)
                            for dc in range(DCH):
                                ps = ps512.tile([128, 512], f32, tag="ps512", name="w2ps")
                                for kc in range(4):
                                    nc.tensor.matmul(ps[:, :w], w2[:, kc, 128 * dc:128 * (dc + 1)],
                                                     g[:, kc, :w], start=(kc == 0), stop=False)
                                nc.tensor.matmul(ps[:, :w], cb2[0:1, 128 * dc:128 * (dc + 1)],
                                                 onesrow[0:1, :w], start=False, stop=True)
                                ds = scrp.tile([128, 512], f32, tag="ds", name="ds2")
                                nc.scalar.copy(ds[:, :w], ps[:, :w])
                                nc.sync.dma_start(arin[128 * dc:128 * (dc + 1), loc0:loc0 + w],
                                                  ds[:, :w])
                            ps = ps512.tile([128, 512], f32, tag="ps512", name="w2row_ps")
                            for kc in range(4):
                                nc.tensor.matmul(ps[0:1, :w], w2row[:, kc:kc + 1],
                                                 g[:, kc, :w], start=(kc == 0), stop=False)
                            nc.tensor.matmul(ps[0:1, :w], sb2[:],
                                             onesrow[0:1, :w], start=False, stop=True)
                            rs = rowp.tile([1, 512], f32, tag="rs", name="rs2")
                            nc.scalar.copy(rs[:, :w], ps[0:1, :w])
                            nc.sync.dma_start(arin[1024:1025, loc0:loc0 + w], rs[:, :w])
                        nc.gpsimd.collective_compute(
                            "AllReduce", A.add, replica_groups=RG,
                            ins=[arin[:].opt()], outs=[arout[:].opt()])
                        for (o0, w) in _half_chunks(b):
                            loc0 = o0 - b * S
                            for dc in range(DCH):
                                ds = scrp.tile([128, 512], f32, tag="ds", name="ds2r")
                                nc.sync.dma_start(ds[:, :w],
                                                  arout[128 * dc:128 * (dc + 1), loc0:loc0 + w])
                                nc.vector.tensor_tensor(h[:, dc, o0:o0 + w],
                                                        h[:, dc, o0:o0 + w],
                                                        ds[:, :w], A.add)
                            rs = rowp.tile([1, 512], f32, tag="rs", name="rs2r")
                            nc.sync.dma_start(rs[:, :w], arout[1024:1025, loc0:loc0 + w])
                            nc.vector.tensor_tensor(colsum[0:1, o0:o0 + w],
                                                    colsum[0:1, o0:o0 + w],
                                                    rs[:, :w], A.add)

                    if DEBUG and l == 0:
                        for dc in range(DCH):
                            nc.sync.dma_start(dbg_h2_d[:, dc, :], h[:, dc, :])

            # ===== final LN + LM head + CE partials =====
            with tc.tile_pool(name="lmpool", bufs=1) as lmp, \
                 tc.tile_pool(name="lmscr", bufs=2) as lms:
                negmrF = lmp.tile([1, TT], bf16, name="negmrF")
                actFs = []
                for (o0, w, c0) in FLN_CHUNKS:
                    act, _ = layer_norm_chunk(o0, w, lms, lmp, act_tag="actF",
                                              act_bufs=4,
                                              negmr_dst=negmrF[0:1, c0:c0 + w])
                    actFs.append(act)
                sume_all = lmp.tile([128, TT // 128, 8], f32, name="sume_all")
                NV = (VC + 511) // 512  # 8 vocab chunks (last 416)
                for vc in range(NV):
                    v0, vw = 512 * vc, min(512, VC - 512 * vc)
                    wlmv = lms.tile([128, DCH, 512], bf16, tag="wlmv", name="wlmv")
                    nc.sync.dma_start(wlmv[:, :, :vw], wlm_d[:, :, v0:v0 + vw])
                    cwv = lms.tile([1, 512], bf16, tag="cwv", name="cwv")
                    nc.sync.dma_start(cwv[:, :vw], cwlm_d[0:1, v0:v0 + vw])
                    cbv = lms.tile([1, 512], bf16, tag="cbv", name="cbv")
                    nc.sync.dma_start(cbv[:, :vw], cblm_d[0:1, v0:v0 + vw])
                    for ci, (o0, w, c0) in enumerate(FLN_CHUNKS):
                        for j in range(w // 128):
                            t0 = c0 + 128 * j   # compact token idx
                            tt = t0 // 128
                            ps = ps512.tile([128, 512], f32, tag="ps512", name="lmps")
                            for dc in range(DCH):
                                nc.tensor.matmul(ps[:, :vw],
                                                 actFs[ci][:, dc, 128 * j:128 * (j + 1)],
                                                 wlmv[:, dc, :vw],
                                                 start=(dc == 0), stop=False)
                            nc.tensor.matmul(ps[:, :vw], negmrF[0:1, t0:t0 + 128],
                                             cwv[0:1, :vw], start=False, stop=False)
                            nc.tensor.matmul(ps[:, :vw], onesrow[0:1, :128],
                                             cbv[0:1, :vw], start=False, stop=True)
                            lg = lms.tile([128, 512], f32, tag="lg", name="lg")
                            nc.scalar.copy(lg[:, :vw], ps[:, :vw])
                            nc.sync.dma_start(logits_d[t0:t0 + 128, v0:v0 + vw],
                                              lg[:, :vw])
                            esc = lms.tile([128, 512], f32, tag="esc", name="esc")
                            nc.scalar.activation(esc[:, :vw], ps[:, :vw], AF.Exp,
                                                 accum_out=sume_all[:, tt, vc:vc + 1])
                for tt in range(TT // 128):
                    se = lms.tile([128, 1], f32, tag="se", name="se")
                    nc.vector.reduce_sum(se[:], sume_all[:, tt, :],
                                         axis=mybir.AxisListType.X)
                    nc.sync.dma_start(sumexp_d[tt:tt + 1, :], se[:])

    nc.compile()
    _cache["sched_entries"] = getattr(tc_ref["tc"], "_perfetto_entries", None)
    return nc


def _prep_inputs(inputs):
    """Host-side: embedding, weight folds, per-core shards -> in_maps."""
    f = lambda a: np.asarray(a, dtype=np.float32)
    x = np.asarray(inputs["x"]).astype(np.int64)
    labels = np.asarray(inputs["labels"]).astype(np.int64)
    ignore_index = int(np.asarray(inputs["ignore_index"]))
    tok_emb, pos_emb = f(inputs["tok_emb"]), f(inputs["pos_emb"])
    mem = f(inputs["mem_embeds"])
    Wq, Wk, Wv, Wo = f(inputs["Wq"]), f(inputs["Wk"]), f(inputs["Wv"]), f(inputs["Wo"])
    W1, b1, W2, b2 = f(inputs["W1"]), f(inputs["b1"]), f(inputs["W2"]), f(inputs["b2"])
    ln1_s, ln1_b = f(inputs["ln1_s"]), f(inputs["ln1_b"])
    ln2_s, ln2_b = f(inputs["ln2_s"]), f(inputs["ln2_b"])
    lnf_s, lnf_b = f(inputs["lnf_s"]), f(inputs["lnf_b"])
    Wlm, blm = f(inputs["Wlm"]), f(inputs["blm"])

    e = tok_emb[x] + pos_emb[:T][None]          # [B, T, D]
    h0 = np.concatenate([mem, e], axis=1)       # [B, S, D]
    h0f = h0.reshape(NTOK, D)
    h0cs = h0f.astype(np.float64).sum(axis=1).astype(np.float32)[None, :]
    h0T = np.ascontiguousarray(h0f.T)           # [D, NTOK]
    h0T = np.ascontiguousarray(h0T.reshape(DCH, 128, NTOK).transpose(1, 0, 2))

    def lhsT_layout(w, kchunks):
        K, Mo = w.shape
        return np.ascontiguousarray(w.reshape(kchunks, 128, Mo).transpose(1, 0, 2))

    bf = lambda a: np.ascontiguousarray(a).astype(_BF16)

    in_maps = []
    for c in range(NC):
        hs = slice(c * DHC, (c + 1) * DHC)
        fs = slice(c * DFC, (c + 1) * DFC)
        vs = slice(c * VC, (c + 1) * VC)
        wq_l, wk_l, wv_l, wo_l, w1_l, w2_l = [], [], [], [], [], []
        worow_l, w2row_l, sb2_l = [], [], []
        cwqkv_l, cbqkv_l, cw1_l, cb1_l, cb2_l = [], [], [], [], []
        for l in range(L):
            s1, bb1 = ln1_s[l], ln1_b[l]
            s2, bb2 = ln2_s[l], ln2_b[l]
            Wq_e = (s1[:, None] * Wq[l][:, hs]) / 8.0
            Wk_e = s1[:, None] * Wk[l][:, hs]
            Wv_e = s1[:, None] * Wv[l][:, hs]
            cw_q = Wq_e.sum(0); cb_q = bb1 @ (Wq[l][:, hs] / 8.0)
            cw_k = Wk_e.sum(0); cb_k = bb1 @ Wk[l][:, hs]
            cw_v = Wv_e.sum(0); cb_v = bb1 @ Wv[l][:, hs]
            Wo_e = Wo[l][hs, :]
            W1_e = s2[:, None] * W1[l][:, fs]
            cw_1 = W1_e.sum(0); cb_1 = bb2 @ W1[l][:, fs] + b1[l][fs]
            W2_e = W2[l][fs, :]
            wq_l.append(lhsT_layout(Wq_e, DCH))
            wk_l.append(lhsT_layout(Wk_e, DCH))
            wv_l.append(lhsT_layout(Wv_e, DCH))
            wo_l.append(Wo_e)
            w1_l.append(lhsT_layout(W1_e, DCH))
            w2_l.append(lhsT_layout(W2_e, 4))
            worow_l.append(Wo_e.sum(1)[:, None])
            w2row_l.append(np.ascontiguousarray(W2_e.sum(1).reshape(4, 128).T))
            sb2_l.append(np.array([[b2[l].sum() / NC]], np.float32))
            cwqkv_l.append(np.concatenate([cw_q, cw_k, cw_v])[None, :])
            cbqkv_l.append(np.concatenate([cb_q, cb_k, cb_v])[None, :])
            cw1_l.append(cw_1[None, :]); cb1_l.append(cb_1[None, :])
            cb2_l.append((b2[l] / NC)[None, :])
        Wlm_e = lnf_s[:, None] * Wlm[:, vs]
        cw_lm = Wlm_e.sum(0); cb_lm = lnf_b @ Wlm[:, vs] + blm[vs]
        in_maps.append({
            "h0": h0T, "h0cs": h0cs,
            "wq": bf(np.stack(wq_l)), "wk": bf(np.stack(wk_l)), "wv": bf(np.stack(wv_l)),
            "wo": bf(np.stack(wo_l)), "w1": bf(np.stack(w1_l)), "w2": bf(np.stack(w2_l)),
            "worow": bf(np.stack(worow_l)), "w2row": bf(np.stack(w2row_l)),
            "sb2": bf(np.stack(sb2_l)),
            "cwqkv": bf(np.stack(cwqkv_l)), "cbqkv": bf(np.stack(cbqkv_l)),
            "cw1": bf(np.stack(cw1_l)), "cb1": bf(np.stack(cb1_l)),
            "cb2": bf(np.stack(cb2_l)),
            "wlm": bf(lhsT_layout(Wlm_e, DCH)),
            "cwlm": bf(cw_lm[None, :]), "cblm": bf(cb_lm[None, :]),
        })
    return in_maps, labels, ignore_index


def run_on_device(inputs, trace=False, trace_kwargs=None):
    from concourse import bass_utils
    if "nc" not in _cache:
        _cache["nc"] = _build()
    nc = _cache["nc"]
    in_maps, labels, ignore_index = _prep_inputs(inputs)
    kw = {}
    if trace:
        kw = dict(trace=True, trace_kwargs=trace_kwargs or {})
    res = bass_utils.run_bass_kernel_spmd(nc, in_maps, core_ids=list(range(NC)), **kw)
    return res, labels, ignore_index


def bench(inputs, iters=6):
    """Time the on-device SPMD executable with inputs pre-staged on device.
    Returns (per_call_seconds_list, first_call_s)."""
    import time
    import jax
    import jax.numpy as jnp
    from jax.sharding import Mesh, PartitionSpec
    from jax.experimental.shard_map import shard_map
    from concourse import bass2jax, mybir
    from concourse.bass2jax import _bass_exec_p, partition_id_tensor

    if "nc" not in _cache:
        _cache["nc"] = _build()
    nc = _cache["nc"]
    bass2jax.install_neuronx_cc_hook()
    in_maps, _, _ = _prep_inputs(inputs)

    partition_name = nc.partition_id_tensor.name if nc.partition_id_tensor else None
    in_names, out_names, out_avals, zero_outs = [], [], [], []
    for alloc in nc.m.functions[0].allocations:
        if not isinstance(alloc, mybir.MemoryLocationSet):
            continue
        name = alloc.memorylocations[0].name
        if alloc.kind == "ExternalInput":
            if name != partition_name:
                in_names.append(name)
        elif alloc.kind == "ExternalOutput":
            out_names.append(name)
            shape = tuple(alloc.tensor_shape)
            dtype = mybir.dt.np(alloc.dtype)
            out_avals.append(jax.core.ShapedArray(shape, dtype))
            zero_outs.append(np.zeros(shape, dtype))
    n_params = len(in_names)
    all_names = list(in_names) + out_names
    if partition_name is not None:
        all_names.append(partition_name)

    def _body(*args):
        operands = list(args)
        if partition_name is not None:
            operands.append(partition_id_tensor())
        return tuple(_bass_exec_p.bind(
            *operands, out_avals=tuple(out_avals), in_names=tuple(all_names),
            out_names=tuple(out_names), lowering_input_output_aliases=(),
            sim_require_finite=True, sim_require_nnan=True, nc=nc))

    devices = jax.devices()[:NC]
    mesh = Mesh(np.asarray(devices), ("core",))
    nin = n_params + len(zero_outs)
    sharded = jax.jit(shard_map(_body, mesh=mesh,
                                in_specs=(PartitionSpec("core"),) * nin,
                                out_specs=(PartitionSpec("core"),) * len(out_names),
                                check_rep=False), keep_unused=True)
    concat_in = [np.concatenate([np.asarray(in_maps[c][nm]) for c in range(NC)], axis=0)
                 for nm in in_names]
    concat_zeros = [np.zeros((NC * z.shape[0], *z.shape[1:]), z.dtype)
                    for z in zero_outs]
    shardings = [jax.sharding.NamedSharding(mesh, PartitionSpec("core"))] * nin
    dev_args = [jax.device_put(a, s) for a, s in zip(concat_in + concat_zeros, shardings)]

    t0 = time.perf_counter()
    out = sharded(*dev_args)
    jax.block_until_ready(out)
    first = time.perf_counter() - t0
    times = []
    for _ in range(iters):
        t0 = time.perf_counter()
        out = sharded(*dev_args)
        jax.block_until_ready(out)
        times.append(time.perf_counter() - t0)
    return times, first


def kernel(**inputs):
    res, labels, ignore_index = run_on_device(inputs)
    shards = [res.results[c] for c in range(NC)]
    logits = np.concatenate([s["logits"].reshape(B, T, VC) for s in shards], axis=2)
    sumexp = np.stack([s["sumexp"].reshape(TT) for s in shards])  # [NC, TT]
    lse = np.log(sumexp.astype(np.float64).sum(axis=0))           # [TT]
    lbl = labels.reshape(-1)
    valid = lbl != ignore_index
    safe = np.where(valid, lbl, 0)
    ll = logits.reshape(TT, V)[np.arange(TT), safe].astype(np.float64)
    nll = lse - ll
    denom = max(int(valid.sum()), 1)
    loss = np.float32((np.where(valid, nll, 0.0).sum()) / denom)
    return logits.astype(np.float32), loss
